# revision 5
# baseline (speedup 1.0000x reference)
"""Trainium2 Bass kernel for nn_CAConvV2 (grouped causal conv + per-tap
feature roll + time mask, output (F, T, L*M, K)).

Self-contained: hardcodes shapes/sharding for
  x: (4, 1024, 512) f32, conv_w: (12288, 1, 3) f32, conv_b: (12288,) f32
  output: (512, 1024, 12, 8) f32

Sharding: 8 cores = 4 feature chunks (128) x 2 time halves (512).
No cross-core communication.

Key structure: the per-(i,l) feature roll is a pure output-index
remapping, so it is applied entirely on the host during assembly.  The
device computes only the depthwise causal conv
    y[g, m, t, il] = b[g,il] + sum_c w[g,il,c] * x[m, t-2+c, g]
as 24 contiguous (128, M, THALF) fp16 slabs (3 fused element passes
each), spread across the ACT/DVE/Pool engines.
"""

import numpy as np

M, T, F = 4, 1024, 512
K, L, CK = 8, 3, 3
NCORES = 8
PCHUNK = 128   # features per core
THALF = 512    # time steps per core
NIL = K * L    # 24 (i,l) channel pairs per feature

# Engine assignment (balanced against the TRN2 cost model; Pool only
# supports plain TensorTensor, not TensorScalarPtr):
#  pass1 (acc = w0*x0 + b): ACT activation, except these ils on DVE ts (4x)
#  pass2 (acc = w1*x1 + acc): for P2_POOL ils: DVE ts product (4x) then
#    Pool tensor_tensor add; else DVE stt (2x)
#  pass3 (slab = w2*x2 + acc): DVE stt
P1_DVE = frozenset((0, 12))
P2_POOL = frozenset((1, 3, 5, 7, 9, 11, 13, 15, 17, 19))

_prog_cache = {}


def _build_program(timing=False):
    from concourse import mybir, bacc
    from concourse.tile import TileContext

    nc = bacc.Bacc("TRN2", target_bir_lowering=False, debug=False,
                   num_devices=NCORES)
    # x_local[g, m, t] = x[m, th*THALF - 2 + t, P*128 + g]  (zeros at t<0)
    x_local = nc.dram_tensor("x_local", (PCHUNK, M, THALF + 2),
                             mybir.dt.float16, kind="ExternalInput")
    # wpack columns: [w0 (24) | w1 (24) | w2 (24) | bias (24)]
    wpack = nc.dram_tensor("wpack", (PCHUNK, 4 * NIL), mybir.dt.float32,
                           kind="ExternalInput")
    out_local = nc.dram_tensor("out_local", (NIL, PCHUNK, M, THALF),
                               mybir.dt.float16,
                               kind="Internal" if timing else "ExternalOutput")
    if timing:
        marker = nc.dram_tensor("marker", (PCHUNK, 1), mybir.dt.float32,
                                kind="ExternalOutput")

    with TileContext(nc) as tc:
        with tc.tile_pool(name="xp", bufs=1) as xpool, \
             tc.tile_pool(name="work", bufs=6) as work, \
             tc.tile_pool(name="stg", bufs=4) as stg:
            wt = xpool.tile([PCHUNK, 4 * NIL], mybir.dt.float32)
            nc.sync.dma_start(out=wt[:], in_=wpack[:, :])
            xt = xpool.tile([PCHUNK, M, THALF + 2], mybir.dt.float16)
            nc.sync.dma_start(out=xt[:], in_=x_local[:, :, :])

            x0 = xt[:, :, 0:THALF]
            x1 = xt[:, :, 1:1 + THALF]
            x2 = xt[:, :, 2:2 + THALF]
            for il in range(NIL):
                w0 = wt[:, il:il + 1]
                w1 = wt[:, NIL + il:NIL + il + 1]
                w2 = wt[:, 2 * NIL + il:2 * NIL + il + 1]
                bb = wt[:, 3 * NIL + il:3 * NIL + il + 1]
                acc1 = work.tile([PCHUNK, M, THALF], mybir.dt.float16,
                                 name="acc1", tag="acc1")
                acc2 = work.tile([PCHUNK, M, THALF], mybir.dt.float16,
                                 name="acc2", tag="acc2")
                slab = stg.tile([PCHUNK, M, THALF], mybir.dt.float16,
                                name="slab", tag="slab")
                # pass1: acc1 = w0*x(t-2) + b
                if il in P1_DVE:
                    nc.vector.tensor_scalar(
                        out=acc1[:], in0=x0, scalar1=w0, scalar2=bb,
                        op0=mybir.AluOpType.mult, op1=mybir.AluOpType.add)
                else:
                    nc.scalar.activation(
                        out=acc1[:], in_=x0,
                        func=mybir.ActivationFunctionType.Identity,
                        scale=w0, bias=bb)
                # pass2: acc2 = w1*x(t-1) + acc1
                if il in P2_POOL:
                    prod = work.tile([PCHUNK, M, THALF], mybir.dt.float16,
                                     name="prod", tag="prod")
                    nc.vector.tensor_scalar(
                        out=prod[:], in0=x1, scalar1=w1, scalar2=None,
                        op0=mybir.AluOpType.mult)
                    nc.gpsimd.tensor_tensor(
                        out=acc2[:], in0=prod[:], in1=acc1[:],
                        op=mybir.AluOpType.add)
                else:
                    nc.vector.scalar_tensor_tensor(
                        out=acc2[:], in0=x1, scalar=w1, in1=acc1[:],
                        op0=mybir.AluOpType.mult, op1=mybir.AluOpType.add)
                # pass3: slab = w2*x(t) + acc2
                nc.vector.scalar_tensor_tensor(
                    out=slab[:], in0=x2, scalar=w2, in1=acc2[:],
                    op0=mybir.AluOpType.mult, op1=mybir.AluOpType.add)
                nc.sync.dma_start(out=out_local[il], in_=slab[:])
            if timing:
                mk = xpool.tile([PCHUNK, 1], mybir.dt.float32, name="mk")
                nc.vector.tensor_copy(out=mk[:], in_=wt[:, 0:1])
                nc.sync.dma_start(out=marker[:, :], in_=mk[:])
    nc.compile()
    return nc


def _build_program_timing():
    return _build_program(timing=True)


def _build_empty_program():
    from concourse import mybir, bacc
    from concourse.tile import TileContext

    nc = bacc.Bacc("TRN2", target_bir_lowering=False, debug=False,
                   num_devices=NCORES)
    din = nc.dram_tensor("dummy_in", (1, 1), mybir.dt.float32,
                         kind="ExternalInput")
    dout = nc.dram_tensor("dummy_out", (1, 1), mybir.dt.float32,
                          kind="ExternalOutput")
    with TileContext(nc) as tc:
        with tc.tile_pool(name="p", bufs=1) as pool:
            t = pool.tile([1, 1], mybir.dt.float32)
            nc.sync.dma_start(out=t[:], in_=din[:, :])
            nc.sync.dma_start(out=dout[:, :], in_=t[:])
    nc.compile()
    return nc


def _prep_inputs(x, conv_w, conv_b):
    """Host-side prep: transpose/pad/cast x per core; pack weights."""
    x = np.asarray(x, dtype=np.float32)
    conv_w = np.asarray(conv_w, dtype=np.float32).reshape(F, NIL, CK)
    conv_b = np.asarray(conv_b, dtype=np.float32).reshape(F, NIL)

    # (M, T+2, F) fp16 with 2 zero rows of left time padding
    xpad = np.zeros((M, T + 2, F), dtype=np.float16)
    xpad[:, 2:] = x.astype(np.float16)

    in_maps = []
    for core in range(NCORES):
        P, th = divmod(core, 2)
        fsl = slice(P * PCHUNK, (P + 1) * PCHUNK)
        # (M, 514, 128) -> (128, M, 514)
        x_loc = np.ascontiguousarray(
            xpad[:, th * THALF:th * THALF + THALF + 2, fsl]
            .transpose(2, 0, 1))
        wp = np.empty((PCHUNK, 4 * NIL), dtype=np.float32)
        wp[:, 0:NIL] = conv_w[fsl, :, 0]
        wp[:, NIL:2 * NIL] = conv_w[fsl, :, 1]
        wp[:, 2 * NIL:3 * NIL] = conv_w[fsl, :, 2]
        wp[:, 3 * NIL:4 * NIL] = conv_b[fsl]
        in_maps.append({"x_local": x_loc, "wpack": wp})
    return in_maps


def _assemble(results):
    """Apply the feature roll (output-index remap), time mask, upcast."""
    full = np.empty((F, T, L * M, K), dtype=np.float32)
    for core in range(NCORES):
        P, th = divmod(core, 2)
        tsl = slice(th * THALF, (th + 1) * THALF)
        blk = results[core]["out_local"].astype(np.float32)
        # blk: (NIL, 128, M, THALF); il = i*L + l
        for i in range(K):
            for l in range(L):
                il = i * L + l
                s = i + l
                rows = (np.arange(P * PCHUNK, (P + 1) * PCHUNK) + s) % F
                # advanced dims (g, m) land in front, sliced t axis last
                full[rows[:, None], tsl,
                     l * M + np.arange(M)[None, :], i] = blk[il]
    # time mask: out[:, t, l*M+m, i] = 0 for t < i + l
    for l in range(L):
        for i in range(K):
            s = i + l
            if s:
                full[:, :s, l * M:(l + 1) * M, i] = 0.0
    return full


def kernel(x, conv_w, conv_b, _want_trace=False):
    from concourse.bass_utils import run_bass_kernel_spmd

    if "nc" not in _prog_cache:
        _prog_cache["nc"] = _build_program()
    nc = _prog_cache["nc"]

    in_maps = _prep_inputs(x, conv_w, conv_b)
    res = run_bass_kernel_spmd(nc, in_maps, core_ids=list(range(NCORES)),
                               trace=_want_trace)
    out = _assemble(res.results)
    if _want_trace:
        return out, res
    return out


# revision 6
# speedup vs baseline: 1.6758x; 1.6758x over previous
"""Trainium2 Bass kernel for nn_CAConvV2 (grouped causal conv + per-tap
feature roll + time mask, output (F, T, L*M, K)).

Self-contained: hardcodes shapes/sharding for
  x: (4, 1024, 512) f32, conv_w: (12288, 1, 3) f32, conv_b: (12288,) f32
  output: (512, 1024, 12, 8) f32

Sharding: 8 cores = 4 feature chunks (128) x 2 time halves (512).
No cross-core communication.

Structure: the per-(i,l) feature roll is a pure output-index remapping,
applied on the host during assembly.  The device computes the depthwise
causal conv  y[g,m,t,il] = b[g,il] + sum_c w[g,il,c] * x[m,t-2+c,g]
as 24 contiguous (128, M, THALF) fp16 slabs, split across engines:

 - PE route (13 ils): 3 taps as diagonal matmuls accumulating in PSUM
   (per m-chunk of 512 cols), ACT reads out PSUM with the bias add.
 - Elementwise route (11 ils): DVE tensor_scalar products (4x fp16
   mode) + tensor_tensor combines on DVE/Pool.

This balances DVE/ACT/Pool/PE/DMA near the cost-model optimum.
"""

import numpy as np

M, T, F = 4, 1024, 512
K, L, CK = 8, 3, 3
NCORES = 8
PCHUNK = 128   # features per core
THALF = 512    # time steps per core
NIL = K * L    # 24 (i,l) channel pairs per feature

# PE-route ils; the rest go elementwise.
PE_SET = frozenset((0, 2, 4, 6, 8, 10, 12, 14, 16, 18, 20, 22, 23))
# Elementwise ils whose first combine runs on Pool (TensorTensor).
POOL_TT1 = frozenset((1, 3, 5, 7, 9, 11, 13, 15, 17))

_prog_cache = {}


def _build_program(timing=False):
    from concourse import mybir, bacc
    from concourse.tile import TileContext

    nc = bacc.Bacc("TRN2", target_bir_lowering=False, debug=False,
                   num_devices=NCORES)
    # x_local[g, m, t] = x[m, th*THALF - 2 + t, P*128 + g]  (zeros at t<0)
    x_local = nc.dram_tensor("x_local", (PCHUNK, M, THALF + 2),
                             mybir.dt.float16, kind="ExternalInput")
    # wpack columns: [w0 (24) | w1 (24) | w2 (24) | bias (24)]
    wpack = nc.dram_tensor("wpack", (PCHUNK, 4 * NIL), mybir.dt.float32,
                           kind="ExternalInput")
    # identity matrix for building diagonal matmul weights
    eye = nc.dram_tensor("eye", (PCHUNK, PCHUNK), mybir.dt.float16,
                         kind="ExternalInput")
    out_local = nc.dram_tensor("out_local", (NIL, PCHUNK, M, THALF),
                               mybir.dt.float16,
                               kind="Internal" if timing else "ExternalOutput")
    if timing:
        marker = nc.dram_tensor("marker", (PCHUNK, 1), mybir.dt.float32,
                                kind="ExternalOutput")

    pe_ils = sorted(PE_SET)

    with TileContext(nc) as tc:
        with tc.tile_pool(name="xp", bufs=1) as xpool, \
             tc.tile_pool(name="work", bufs=10) as work, \
             tc.tile_pool(name="stg", bufs=5) as stg, \
             tc.psum_pool(name="ps", bufs=8) as psp:
            wt = xpool.tile([PCHUNK, 4 * NIL], mybir.dt.float32)
            nc.sync.dma_start(out=wt[:], in_=wpack[:, :])
            em = xpool.tile([PCHUNK, PCHUNK], mybir.dt.float16)
            nc.sync.dma_start(out=em[:], in_=eye[:, :])
            xt = xpool.tile([PCHUNK, M, THALF + 2], mybir.dt.float16)
            nc.sync.dma_start(out=xt[:], in_=x_local[:, :, :])
            # all diagonal weight matrices, built once on DVE (cheap 4x ts)
            dtile = xpool.tile([PCHUNK, len(pe_ils) * CK * PCHUNK],
                               mybir.dt.float16, name="dtile")

            def emit_pe(il, dbase):
                for c in range(CK):
                    dsl = dtile[:, (dbase + c) * PCHUNK:
                                (dbase + c + 1) * PCHUNK]
                    nc.vector.tensor_scalar(
                        out=dsl, in0=em[:],
                        scalar1=wt[:, c * NIL + il:c * NIL + il + 1],
                        scalar2=None, op0=mybir.AluOpType.mult)
                bb = wt[:, 3 * NIL + il:3 * NIL + il + 1]
                slab = stg.tile([PCHUNK, M, THALF], mybir.dt.float16,
                                name="slab", tag="slab")
                for m in range(M):
                    ps = psp.tile([PCHUNK, THALF], mybir.dt.float32,
                                  name="ps", tag="ps")
                    for c in range(CK):
                        dsl = dtile[:, (dbase + c) * PCHUNK:
                                    (dbase + c + 1) * PCHUNK]
                        nc.tensor.matmul(
                            out=ps[:], lhsT=dsl, rhs=xt[:, m, c:c + THALF],
                            start=(c == 0), stop=(c == CK - 1))
                    nc.scalar.activation(
                        out=slab[:, m, :], in_=ps[:],
                        func=mybir.ActivationFunctionType.Identity,
                        scale=1.0, bias=bb)
                nc.sync.dma_start(out=out_local[il], in_=slab[:])

            def emit_ew(il):
                w0 = wt[:, il:il + 1]
                w1 = wt[:, NIL + il:NIL + il + 1]
                w2 = wt[:, 2 * NIL + il:2 * NIL + il + 1]
                bb = wt[:, 3 * NIL + il:3 * NIL + il + 1]
                x0 = xt[:, :, 0:THALF]
                x1 = xt[:, :, 1:1 + THALF]
                x2 = xt[:, :, 2:2 + THALF]
                p0 = work.tile([PCHUNK, M, THALF], mybir.dt.float16,
                               name="p0", tag="p0")
                p1 = work.tile([PCHUNK, M, THALF], mybir.dt.float16,
                               name="p1", tag="p1")
                p2 = work.tile([PCHUNK, M, THALF], mybir.dt.float16,
                               name="p2", tag="p2")
                t1 = work.tile([PCHUNK, M, THALF], mybir.dt.float16,
                               name="t1", tag="t1")
                slab = stg.tile([PCHUNK, M, THALF], mybir.dt.float16,
                                name="slab", tag="slab")
                nc.vector.tensor_scalar(
                    out=p0[:], in0=x0, scalar1=w0, scalar2=bb,
                    op0=mybir.AluOpType.mult, op1=mybir.AluOpType.add)
                nc.vector.tensor_scalar(
                    out=p1[:], in0=x1, scalar1=w1, scalar2=None,
                    op0=mybir.AluOpType.mult)
                nc.vector.tensor_scalar(
                    out=p2[:], in0=x2, scalar1=w2, scalar2=None,
                    op0=mybir.AluOpType.mult)
                eng1 = nc.gpsimd if il in POOL_TT1 else nc.vector
                eng1.tensor_tensor(out=t1[:], in0=p0[:], in1=p1[:],
                                   op=mybir.AluOpType.add)
                nc.vector.tensor_tensor(out=slab[:], in0=t1[:], in1=p2[:],
                                        op=mybir.AluOpType.add)
                nc.sync.dma_start(out=out_local[il], in_=slab[:])

            dbase_of = {il: i * CK for i, il in enumerate(pe_ils)}
            for il in range(NIL):
                if il in PE_SET:
                    emit_pe(il, dbase_of[il])
                else:
                    emit_ew(il)
            if timing:
                mk = xpool.tile([PCHUNK, 1], mybir.dt.float32, name="mk")
                nc.vector.tensor_copy(out=mk[:], in_=wt[:, 0:1])
                nc.sync.dma_start(out=marker[:, :], in_=mk[:])
    nc.compile()
    return nc


def _build_program_timing():
    return _build_program(timing=True)


def _build_empty_program():
    from concourse import mybir, bacc
    from concourse.tile import TileContext

    nc = bacc.Bacc("TRN2", target_bir_lowering=False, debug=False,
                   num_devices=NCORES)
    din = nc.dram_tensor("dummy_in", (1, 1), mybir.dt.float32,
                         kind="ExternalInput")
    dout = nc.dram_tensor("dummy_out", (1, 1), mybir.dt.float32,
                          kind="ExternalOutput")
    with TileContext(nc) as tc:
        with tc.tile_pool(name="p", bufs=1) as pool:
            t = pool.tile([1, 1], mybir.dt.float32)
            nc.sync.dma_start(out=t[:], in_=din[:, :])
            nc.sync.dma_start(out=dout[:, :], in_=t[:])
    nc.compile()
    return nc


def _prep_inputs(x, conv_w, conv_b):
    """Host-side prep: transpose/pad/cast x per core; pack weights."""
    x = np.asarray(x, dtype=np.float32)
    conv_w = np.asarray(conv_w, dtype=np.float32).reshape(F, NIL, CK)
    conv_b = np.asarray(conv_b, dtype=np.float32).reshape(F, NIL)

    # (M, T+2, F) fp16 with 2 zero rows of left time padding
    xpad = np.zeros((M, T + 2, F), dtype=np.float16)
    xpad[:, 2:] = x.astype(np.float16)
    eye = np.eye(PCHUNK, dtype=np.float16)

    in_maps = []
    for core in range(NCORES):
        P, th = divmod(core, 2)
        fsl = slice(P * PCHUNK, (P + 1) * PCHUNK)
        # (M, 514, 128) -> (128, M, 514)
        x_loc = np.ascontiguousarray(
            xpad[:, th * THALF:th * THALF + THALF + 2, fsl]
            .transpose(2, 0, 1))
        wp = np.empty((PCHUNK, 4 * NIL), dtype=np.float32)
        wp[:, 0:NIL] = conv_w[fsl, :, 0]
        wp[:, NIL:2 * NIL] = conv_w[fsl, :, 1]
        wp[:, 2 * NIL:3 * NIL] = conv_w[fsl, :, 2]
        wp[:, 3 * NIL:4 * NIL] = conv_b[fsl]
        in_maps.append({"x_local": x_loc, "wpack": wp, "eye": eye})
    return in_maps


def _assemble(results):
    """Apply the feature roll (output-index remap), time mask, upcast."""
    full = np.empty((F, T, L * M, K), dtype=np.float32)
    mcols = np.arange(M)[None, :]
    for core in range(NCORES):
        P, th = divmod(core, 2)
        tsl = slice(th * THALF, (th + 1) * THALF)
        blk = results[core]["out_local"].astype(np.float32)
        # blk: (NIL, 128, M, THALF); il = i*L + l
        for i in range(K):
            for l in range(L):
                il = i * L + l
                s = i + l
                rows = (np.arange(P * PCHUNK, (P + 1) * PCHUNK) + s) % F
                # advanced dims (g, m) land in front, sliced t axis last
                full[rows[:, None], tsl, l * M + mcols, i] = blk[il]
    # time mask: out[:, t, l*M+m, i] = 0 for t < i + l
    for l in range(L):
        for i in range(K):
            s = i + l
            if s:
                full[:, :s, l * M:(l + 1) * M, i] = 0.0
    return full


def kernel(x, conv_w, conv_b, _want_trace=False):
    from concourse.bass_utils import run_bass_kernel_spmd

    if "nc" not in _prog_cache:
        _prog_cache["nc"] = _build_program()
    nc = _prog_cache["nc"]

    in_maps = _prep_inputs(x, conv_w, conv_b)
    res = run_bass_kernel_spmd(nc, in_maps, core_ids=list(range(NCORES)),
                               trace=_want_trace)
    out = _assemble(res.results)
    if _want_trace:
        return out, res
    return out


# revision 8
# speedup vs baseline: 1.8477x; 1.1026x over previous
"""Trainium2 Bass kernel for nn_CAConvV2 (grouped causal conv + per-tap
feature roll + time mask, output (F, T, L*M, K)).

Self-contained: hardcodes shapes/sharding for
  x: (4, 1024, 512) f32, conv_w: (12288, 1, 3) f32, conv_b: (12288,) f32
  output: (512, 1024, 12, 8) f32

Sharding: 8 cores = 4 feature chunks (128) x 2 time halves (512).
No cross-core communication.

Structure: the per-(i,l) feature roll is a pure output-index remapping,
applied on the host during assembly.  The device computes the depthwise
causal conv  y[g,m,t,il] = b[g,il] + sum_c w[g,il,c] * x[m,t-2+c,g]
as 24 contiguous (128, M, THALF) fp16 slabs, split across engines:

 - PE route (13 ils): 3 taps as diagonal matmuls accumulating in PSUM
   (per m-chunk of 512 cols), ACT reads out PSUM with the bias add.
 - Elementwise route (11 ils): DVE tensor_scalar products (4x fp16
   mode) + tensor_tensor combines on DVE/Pool.

This balances DVE/ACT/Pool/PE/DMA near the cost-model optimum.
"""

import numpy as np

M, T, F = 4, 1024, 512
K, L, CK = 8, 3, 3
NCORES = 8
PCHUNK = 128   # features per core
THALF = 512    # time steps per core
NIL = K * L    # 24 (i,l) channel pairs per feature

# PE-route ils; the rest go elementwise.
PE_SET = frozenset((0, 2, 4, 6, 8, 10, 12, 14, 16, 18, 20, 21, 22, 23))
# Elementwise ils whose first combine runs on Pool (TensorTensor).
POOL_TT1 = frozenset((1, 3, 5, 7, 9, 11, 13, 15))

_prog_cache = {}


def _build_program(timing=False):
    from concourse import mybir, bacc
    from concourse.tile import TileContext

    nc = bacc.Bacc("TRN2", target_bir_lowering=False, debug=False,
                   num_devices=NCORES)
    # x_local[g, m, t] = x[m, th*THALF - 2 + t, P*128 + g]  (zeros at t<0)
    x_local = nc.dram_tensor("x_local", (PCHUNK, M, THALF + 2),
                             mybir.dt.float16, kind="ExternalInput")
    # wpack columns: [w0 (24) | w1 (24) | w2 (24) | bias (24)]
    wpack = nc.dram_tensor("wpack", (PCHUNK, 4 * NIL), mybir.dt.float32,
                           kind="ExternalInput")
    # identity matrix for building diagonal matmul weights
    eye = nc.dram_tensor("eye", (PCHUNK, PCHUNK), mybir.dt.float16,
                         kind="ExternalInput")
    out_local = nc.dram_tensor("out_local", (NIL, PCHUNK, M, THALF),
                               mybir.dt.float16,
                               kind="Internal" if timing else "ExternalOutput")
    if timing:
        marker = nc.dram_tensor("marker", (PCHUNK, 1), mybir.dt.float32,
                                kind="ExternalOutput")

    pe_ils = sorted(PE_SET)

    ew_ils = sorted(set(range(NIL)) - PE_SET)

    with TileContext(nc) as tc:
        with tc.tile_pool(name="xp", bufs=1) as xpool, \
             tc.tile_pool(name="wka", bufs=4) as wka, \
             tc.tile_pool(name="wkb", bufs=11) as wkb, \
             tc.tile_pool(name="stg", bufs=6) as stg, \
             tc.psum_pool(name="ps", bufs=8) as psp:
            wt = xpool.tile([PCHUNK, 4 * NIL], mybir.dt.float32)
            nc.sync.dma_start(out=wt[:], in_=wpack[:, :])
            em = xpool.tile([PCHUNK, PCHUNK], mybir.dt.float16)
            nc.sync.dma_start(out=em[:], in_=eye[:, :])
            xt = xpool.tile([PCHUNK, M, THALF + 2], mybir.dt.float16)
            nc.sync.dma_start(out=xt[:], in_=x_local[:, :, :])
            # all diagonal weight matrices, built on DVE (cheap 4x ts)
            dtile = xpool.tile([PCHUNK, len(pe_ils) * CK * PCHUNK],
                               mybir.dt.float16, name="dtile")
            # prewarm the ACT function table while ACT is otherwise idle
            warm = xpool.tile([PCHUNK, 1], mybir.dt.float32, name="warm")
            nc.scalar.activation(out=warm[:], in_=wt[:, 0:1],
                                 func=mybir.ActivationFunctionType.Identity,
                                 scale=1.0, bias=0.0)

            dbase_of = {il: i * CK for i, il in enumerate(pe_ils)}

            def emit_diags(il):
                dbase = dbase_of[il]
                for c in range(CK):
                    dsl = dtile[:, (dbase + c) * PCHUNK:
                                (dbase + c + 1) * PCHUNK]
                    nc.vector.tensor_scalar(
                        out=dsl, in0=em[:],
                        scalar1=wt[:, c * NIL + il:c * NIL + il + 1],
                        scalar2=None, op0=mybir.AluOpType.mult)

            def emit_pe(il):
                dbase = dbase_of[il]
                bb = wt[:, 3 * NIL + il:3 * NIL + il + 1]
                slab = stg.tile([PCHUNK, M, THALF], mybir.dt.float16,
                                name="slab", tag="slab")
                for m in range(M):
                    ps = psp.tile([PCHUNK, THALF], mybir.dt.float32,
                                  name="ps", tag="ps")
                    for c in range(CK):
                        dsl = dtile[:, (dbase + c) * PCHUNK:
                                    (dbase + c + 1) * PCHUNK]
                        nc.tensor.matmul(
                            out=ps[:], lhsT=dsl, rhs=xt[:, m, c:c + THALF],
                            start=(c == 0), stop=(c == CK - 1))
                    nc.scalar.activation(
                        out=slab[:, m, :], in_=ps[:],
                        func=mybir.ActivationFunctionType.Identity,
                        scale=1.0, bias=bb)
                nc.sync.dma_start(out=out_local[il], in_=slab[:])

            # elementwise pairs, stage-split for software pipelining
            ew_state = {}

            def emit_ew_products(il):
                w0 = wt[:, il:il + 1]
                w1 = wt[:, NIL + il:NIL + il + 1]
                bb = wt[:, 3 * NIL + il:3 * NIL + il + 1]
                p0 = wka.tile([PCHUNK, M, THALF], mybir.dt.float16,
                              name="p0", tag="p0")
                p1 = wka.tile([PCHUNK, M, THALF], mybir.dt.float16,
                              name="p1", tag="p1")
                nc.vector.tensor_scalar(
                    out=p0[:], in0=xt[:, :, 0:THALF], scalar1=w0, scalar2=bb,
                    op0=mybir.AluOpType.mult, op1=mybir.AluOpType.add)
                nc.vector.tensor_scalar(
                    out=p1[:], in0=xt[:, :, 1:1 + THALF], scalar1=w1,
                    scalar2=None, op0=mybir.AluOpType.mult)
                ew_state[il] = (p0, p1)

            def emit_ew_tt1(il):
                p0, p1 = ew_state[il]
                t1 = wkb.tile([PCHUNK, M, THALF], mybir.dt.float16,
                              name="t1", tag="t1")
                eng1 = nc.gpsimd if il in POOL_TT1 else nc.vector
                eng1.tensor_tensor(out=t1[:], in0=p0[:], in1=p1[:],
                                   op=mybir.AluOpType.add)
                ew_state[il] = t1

            def emit_ew_tail(il):
                w2 = wt[:, 2 * NIL + il:2 * NIL + il + 1]
                t1 = ew_state.pop(il)
                p2 = wka.tile([PCHUNK, M, THALF], mybir.dt.float16,
                              name="p2", tag="p2")
                slab = stg.tile([PCHUNK, M, THALF], mybir.dt.float16,
                                name="slab", tag="slab")
                nc.vector.tensor_scalar(
                    out=p2[:], in0=xt[:, :, 2:2 + THALF], scalar1=w2,
                    scalar2=None, op0=mybir.AluOpType.mult)
                nc.vector.tensor_tensor(out=slab[:], in0=t1[:], in1=p2[:],
                                        op=mybir.AluOpType.add)
                nc.sync.dma_start(out=out_local[il], in_=slab[:])

            # Software-pipelined emission: diags/products run 2 rounds
            # ahead of consumers; Pool-fed combines complete 3 rounds later.
            NR = len(pe_ils)
            emit_diags(pe_ils[0])
            emit_diags(pe_ils[1])
            emit_ew_products(ew_ils[0])
            emit_ew_products(ew_ils[1])
            for r in range(NR):
                emit_pe(pe_ils[r])
                if r + 2 < NR:
                    emit_diags(pe_ils[r + 2])
                if r + 2 < len(ew_ils):
                    emit_ew_products(ew_ils[r + 2])
                if r < len(ew_ils):
                    emit_ew_tt1(ew_ils[r])
                if r >= 3 and r - 3 < len(ew_ils):
                    emit_ew_tail(ew_ils[r - 3])
            for j in range(max(0, NR - 3), len(ew_ils)):
                emit_ew_tail(ew_ils[j])
            if timing:
                mk = xpool.tile([PCHUNK, 1], mybir.dt.float32, name="mk")
                nc.vector.tensor_copy(out=mk[:], in_=wt[:, 0:1])
                nc.sync.dma_start(out=marker[:, :], in_=mk[:])
    nc.compile()
    return nc


def _build_program_timing():
    return _build_program(timing=True)


def _build_empty_program():
    from concourse import mybir, bacc
    from concourse.tile import TileContext

    nc = bacc.Bacc("TRN2", target_bir_lowering=False, debug=False,
                   num_devices=NCORES)
    din = nc.dram_tensor("dummy_in", (1, 1), mybir.dt.float32,
                         kind="ExternalInput")
    dout = nc.dram_tensor("dummy_out", (1, 1), mybir.dt.float32,
                          kind="ExternalOutput")
    with TileContext(nc) as tc:
        with tc.tile_pool(name="p", bufs=1) as pool:
            t = pool.tile([1, 1], mybir.dt.float32)
            nc.sync.dma_start(out=t[:], in_=din[:, :])
            nc.sync.dma_start(out=dout[:, :], in_=t[:])
    nc.compile()
    return nc


def _prep_inputs(x, conv_w, conv_b):
    """Host-side prep: transpose/pad/cast x per core; pack weights."""
    x = np.asarray(x, dtype=np.float32)
    conv_w = np.asarray(conv_w, dtype=np.float32).reshape(F, NIL, CK)
    conv_b = np.asarray(conv_b, dtype=np.float32).reshape(F, NIL)

    # (M, T+2, F) fp16 with 2 zero rows of left time padding
    xpad = np.zeros((M, T + 2, F), dtype=np.float16)
    xpad[:, 2:] = x.astype(np.float16)
    eye = np.eye(PCHUNK, dtype=np.float16)

    in_maps = []
    for core in range(NCORES):
        P, th = divmod(core, 2)
        fsl = slice(P * PCHUNK, (P + 1) * PCHUNK)
        # (M, 514, 128) -> (128, M, 514)
        x_loc = np.ascontiguousarray(
            xpad[:, th * THALF:th * THALF + THALF + 2, fsl]
            .transpose(2, 0, 1))
        wp = np.empty((PCHUNK, 4 * NIL), dtype=np.float32)
        wp[:, 0:NIL] = conv_w[fsl, :, 0]
        wp[:, NIL:2 * NIL] = conv_w[fsl, :, 1]
        wp[:, 2 * NIL:3 * NIL] = conv_w[fsl, :, 2]
        wp[:, 3 * NIL:4 * NIL] = conv_b[fsl]
        in_maps.append({"x_local": x_loc, "wpack": wp, "eye": eye})
    return in_maps


def _assemble(results):
    """Apply the feature roll (output-index remap), time mask, upcast."""
    full = np.empty((F, T, L * M, K), dtype=np.float32)
    mcols = np.arange(M)[None, :]
    for core in range(NCORES):
        P, th = divmod(core, 2)
        tsl = slice(th * THALF, (th + 1) * THALF)
        blk = results[core]["out_local"].astype(np.float32)
        # blk: (NIL, 128, M, THALF); il = i*L + l
        for i in range(K):
            for l in range(L):
                il = i * L + l
                s = i + l
                rows = (np.arange(P * PCHUNK, (P + 1) * PCHUNK) + s) % F
                # advanced dims (g, m) land in front, sliced t axis last
                full[rows[:, None], tsl, l * M + mcols, i] = blk[il]
    # time mask: out[:, t, l*M+m, i] = 0 for t < i + l
    for l in range(L):
        for i in range(K):
            s = i + l
            if s:
                full[:, :s, l * M:(l + 1) * M, i] = 0.0
    return full


def kernel(x, conv_w, conv_b, _want_trace=False):
    from concourse.bass_utils import run_bass_kernel_spmd

    if "nc" not in _prog_cache:
        _prog_cache["nc"] = _build_program()
    nc = _prog_cache["nc"]

    in_maps = _prep_inputs(x, conv_w, conv_b)
    res = run_bass_kernel_spmd(nc, in_maps, core_ids=list(range(NCORES)),
                               trace=_want_trace)
    out = _assemble(res.results)
    if _want_trace:
        return out, res
    return out
